# revision 11
# baseline (speedup 1.0000x reference)
"""Trainium2 Bass kernel for nn_AsymmetricTreeLayer.

Computes out[b, j] = y[b, 2j] + y[b, 2j+1] where
  y = leaky_relu(inputs * kernel, 0.01) + inputs * copy_placer
for inputs (262144, 256) f32, kernel/copy_placer (1, 256) f32.

Strategy: pure data parallelism over 8 NeuronCores (batch sharded,
32768 rows per core). Per core the problem is memory bound
(32 MB in + 16 MB out at ~360 GB/s/core). Inputs are cast f32->bf16
during the DMA load (SWDGE cast), the per-column scale multiply runs on
VectorE at bf16 2x rate, leaky-relu on ScalarE (Lrelu activation), and
the pairwise reduction is a stride-2 tensor_tensor add writing f32.

Raw Bass (no Tile framework): this toolchain's walrus accepts at most
ONE sync-wait command per instruction, so all synchronization is manual
with standalone wait_ge instructions where more than one condition must
hold.
"""

import numpy as np
import sys

try:
    import concourse.bass as bass
except ImportError:  # staged repo location in the container
    sys.path.insert(0, "/opt/trn_rl_repo")
    import concourse.bass as bass
import ml_dtypes
import concourse.mybir as mybir
from concourse.bass_utils import run_bass_kernel_spmd

NCORES = 8
BATCH = 262144
L = 256
SHARD = BATCH // NCORES          # 32768 rows per core
P = 128                          # SBUF partitions
FD = 4096                        # f32 elems per partition per tile
ROWS_PER_PART = FD // L          # 16 rows of the (SHARD, L) shard per partition
NT = SHARD * L // (P * FD)       # 16 tiles per core
ALPHA = 0.01

BX = 3   # xb (input) buffer slots
BZ = 2   # z buffer slots
BY = 2   # y buffer slots
BO = 3   # output buffer slots


def _build_lrelu():
    """Fast path (copy_placer == 0): mul on DVE, Lrelu on ACT, pairwise on DVE."""
    nc = bass.Bass()
    x = nc.declare_dram_parameter("x", [NT, P, FD], mybir.dt.float32, isOutput=False)
    kk = nc.declare_dram_parameter("kk", [P, FD], mybir.dt.bfloat16, isOutput=False)
    out = nc.declare_dram_parameter("out", [NT, P, FD // 2], mybir.dt.float32, isOutput=True)

    k_sb = nc.alloc_sbuf_tensor("k_sb", [P, FD], mybir.dt.bfloat16).ap()
    xb = [nc.alloc_sbuf_tensor(f"xb{s}", [P, FD], mybir.dt.bfloat16).ap() for s in range(BX)]
    zb = [nc.alloc_sbuf_tensor(f"zb{s}", [P, FD], mybir.dt.bfloat16).ap() for s in range(BZ)]
    yb = [nc.alloc_sbuf_tensor(f"yb{s}", [P, FD], mybir.dt.bfloat16).ap() for s in range(BY)]
    ob = [nc.alloc_sbuf_tensor(f"ob{s}", [P, FD // 2], mybir.dt.float32).ap() for s in range(BO)]

    ksem = nc.alloc_semaphore("ksem")
    lsem = [nc.alloc_semaphore(f"lsem{s}") for s in range(BX)]
    osem = [nc.alloc_semaphore(f"osem{s}") for s in range(BO)]
    vz = nc.alloc_semaphore("vz")    # +1 per DVE mul
    ay = nc.alloc_semaphore("ay")    # +1 per ACT lrelu
    vo = nc.alloc_semaphore("vo")    # +1 per DVE pairwise

    with nc.Block() as block:

        @block.gpsimd
        def _(g):
            g.dma_start(out=k_sb, in_=kk[:]).then_inc(ksem, 16)
            for t in range(NT):
                if t >= BX:
                    # WAR: slot reused; its previous mul must be done
                    g.wait_ge(vz, t - BX + 1)
                g.dma_start(out=xb[t % BX], in_=x[t]).then_inc(lsem[t % BX], 16)

        @block.vector
        def _(v):
            def pairwise(j):
                if j >= BO:
                    v.wait_ge(osem[j % BO], 16 * (j // BO))
                y = yb[j % BY]
                v.tensor_add(ob[j % BO], y[:, 0::2], y[:, 1::2])._wait_ge(
                    ay, j + 1
                ).then_inc(vo, 1)

            v.wait_ge(ksem, 16)
            for t in range(NT):
                if t >= BZ:
                    v.wait_ge(ay, t - BZ + 1)  # WAR: lrelu done reading z slot
                v.tensor_mul(zb[t % BZ], xb[t % BX], k_sb)._wait_ge(
                    lsem[t % BX], 16 * (t // BX + 1)
                ).then_inc(vz, 1)
                if t > 0:
                    pairwise(t - 1)
            pairwise(NT - 1)

        @block.scalar
        def _(a):
            for t in range(NT):
                if t >= BY:
                    a.wait_ge(vo, t - BY + 1)  # WAR: pairwise done reading y slot
                a.activation(
                    yb[t % BY], zb[t % BZ], mybir.ActivationFunctionType.Lrelu,
                    bias=0.0, scale=1.0, alpha=ALPHA,
                )._wait_ge(vz, t + 1).then_inc(ay, 1)

        @block.sync
        def _(s):
            for t in range(NT):
                s.dma_start(out=out[t], in_=ob[t % BO])._wait_ge(vo, t + 1).then_inc(
                    osem[t % BO], 16
                )
            # drain: make sure every output DMA has landed before exit
            for sl in range(BO):
                n_dmas = NT // BO + (1 if sl < NT % BO else 0)
                s.wait_ge(osem[sl], 16 * n_dmas)

    nc.finalize()
    return nc


def _build_max(general: bool):
    """Fallback/general path on DVE only:
    out = pairwise(max(x*k_hi, x*k_lo)) with
      k_hi = kernel + copy_placer, k_lo = alpha*kernel + copy_placer.
    When general=False, k_lo tile is not loaded and b = a * alpha
    (valid for copy_placer == 0) via tensor_scalar at 4x rate.
    """
    nc = bass.Bass()
    x = nc.declare_dram_parameter("x", [NT, P, FD], mybir.dt.float32, isOutput=False)
    khi = nc.declare_dram_parameter("khi", [P, FD], mybir.dt.bfloat16, isOutput=False)
    if general:
        klo = nc.declare_dram_parameter("klo", [P, FD], mybir.dt.bfloat16, isOutput=False)
    out = nc.declare_dram_parameter("out", [NT, P, FD // 2], mybir.dt.float32, isOutput=True)

    khi_sb = nc.alloc_sbuf_tensor("khi_sb", [P, FD], mybir.dt.bfloat16).ap()
    if general:
        klo_sb = nc.alloc_sbuf_tensor("klo_sb", [P, FD], mybir.dt.bfloat16).ap()
    xb = [nc.alloc_sbuf_tensor(f"xb{s}", [P, FD], mybir.dt.bfloat16).ap() for s in range(BX)]
    ab = [nc.alloc_sbuf_tensor(f"ab{s}", [P, FD], mybir.dt.bfloat16).ap() for s in range(2)]
    bb = [nc.alloc_sbuf_tensor(f"bb{s}", [P, FD], mybir.dt.bfloat16).ap() for s in range(2)]
    yb = [nc.alloc_sbuf_tensor(f"yb{s}", [P, FD], mybir.dt.bfloat16).ap() for s in range(2)]
    ob = [nc.alloc_sbuf_tensor(f"ob{s}", [P, FD // 2], mybir.dt.float32).ap() for s in range(BO)]

    ksem = nc.alloc_semaphore("ksem")
    lsem = [nc.alloc_semaphore(f"lsem{s}") for s in range(BX)]
    osem = [nc.alloc_semaphore(f"osem{s}") for s in range(BO)]
    vz = nc.alloc_semaphore("vz")    # +1 per a-mul (for load WAR)
    vo = nc.alloc_semaphore("vo")    # +1 per pairwise

    with nc.Block() as block:

        @block.gpsimd
        def _(g):
            g.dma_start(out=khi_sb, in_=khi[:]).then_inc(ksem, 16)
            if general:
                g.dma_start(out=klo_sb, in_=klo[:]).then_inc(ksem, 16)
            for t in range(NT):
                if t >= BX:
                    g.wait_ge(vz, t - BX + 1)
                g.dma_start(out=xb[t % BX], in_=x[t]).then_inc(lsem[t % BX], 16)

        @block.vector
        def _(v):
            v.wait_ge(ksem, 32 if general else 16)
            for t in range(NT):
                a, b, y = ab[t % 2], bb[t % 2], yb[t % 2]
                v.tensor_mul(a, xb[t % BX], khi_sb)._wait_ge(
                    lsem[t % BX], 16 * (t // BX + 1)
                ).then_inc(vz, 1)
                if general:
                    v.tensor_mul(b, xb[t % BX], klo_sb)
                else:
                    v.tensor_scalar_mul(b, a, ALPHA)
                v.tensor_max(y, a, b)
                if t >= BO:
                    v.wait_ge(osem[t % BO], 16 * (t // BO))
                v.tensor_add(ob[t % BO], y[:, 0::2], y[:, 1::2]).then_inc(vo, 1)

        @block.sync
        def _(s):
            for t in range(NT):
                s.dma_start(out=out[t], in_=ob[t % BO])._wait_ge(vo, t + 1).then_inc(
                    osem[t % BO], 16
                )
            for sl in range(BO):
                n_dmas = NT // BO + (1 if sl < NT % BO else 0)
                s.wait_ge(osem[sl], 16 * n_dmas)

    nc.finalize()
    return nc


FT = 4096                        # max batch elems per partition per tile
# tile sizes along the batch axis; tapered tail shortens the drain chain
TILES = [4096] * 7 + [2048] * 2
assert sum(TILES) == SHARD
# tiles whose odd-half leaky runs on VectorE (max-trick) instead of ScalarE,
# balancing the two engines so neither paces the pipeline tail
DVE_YB = {1, 3, 5, 7, 8}
SA = 3                           # input (A/B) buffer slots
SY = 2                           # ya/yb buffer slots
SO = 3                           # output buffer slots


def _build_transposed():
    """Transposed fast path (copy_placer == 0).

    Host pre-transposes each core's shard to column-major bf16 and splits
    even/odd columns: xe[j, b] = x[b, 2j], xo[j, b] = x[b, 2j+1].
    Then k becomes a per-partition scale, so ScalarE computes
    y = Lrelu(k_j * x) in a single activation op per half, and VectorE
    only does out[j, b] = ye + yo (contiguous bf16 add at 2x rate).
    Output is bf16 (128, SHARD), transposed back and upcast on the host.
    """
    nc = bass.Bass()
    xe = nc.declare_dram_parameter("xe", [P, SHARD], mybir.dt.bfloat16, isOutput=False)
    xo = nc.declare_dram_parameter("xo", [P, SHARD], mybir.dt.bfloat16, isOutput=False)
    kk = nc.declare_dram_parameter("kk", [P, 2], mybir.dt.float32, isOutput=False)
    out = nc.declare_dram_parameter("out", [P, SHARD], mybir.dt.bfloat16, isOutput=True)

    NTILES = len(TILES)
    OFFS = [sum(TILES[:i]) for i in range(NTILES)]

    kk_sb = nc.alloc_sbuf_tensor("kk_sb", [P, 2], mybir.dt.float32).ap()
    A = [nc.alloc_sbuf_tensor(f"A{s}", [P, FT], mybir.dt.bfloat16).ap() for s in range(SA)]
    B = [nc.alloc_sbuf_tensor(f"B{s}", [P, FT], mybir.dt.bfloat16).ap() for s in range(SA)]
    ya = [nc.alloc_sbuf_tensor(f"ya{s}", [P, FT], mybir.dt.bfloat16).ap() for s in range(SY)]
    yb = [nc.alloc_sbuf_tensor(f"yb{s}", [P, FT], mybir.dt.bfloat16).ap() for s in range(SY)]
    ob = [nc.alloc_sbuf_tensor(f"ob{s}", [P, FT], mybir.dt.bfloat16).ap() for s in range(SO)]
    tmp = nc.alloc_sbuf_tensor("tmp", [P, FT], mybir.dt.bfloat16).ap()

    ksem = nc.alloc_semaphore("ksem")
    lsa = [nc.alloc_semaphore(f"lsa{s}") for s in range(SA)]
    lsb = [nc.alloc_semaphore(f"lsb{s}") for s in range(SA)]
    osem = [nc.alloc_semaphore(f"osem{s}") for s in range(SO)]
    ay = nc.alloc_semaphore("ay")    # +1 per ya (ScalarE)
    byd = nc.alloc_semaphore("byd")  # +1 per yb, whichever engine produced it
    vo = nc.alloc_semaphore("vo")    # +1 per DVE pairwise add

    with nc.Block() as block:

        @block.gpsimd
        def _(g):
            for t in range(NTILES):
                s = t % SA
                sl = slice(OFFS[t], OFFS[t] + TILES[t])
                if t >= SA:
                    g.wait_ge(ay, t - SA + 1)   # ya(t-SA) done reading A slot
                g.dma_start(out=A[s][:, :TILES[t]], in_=xe[:, sl]).then_inc(lsa[s], 16)
                if t >= SA:
                    g.wait_ge(byd, t - SA + 1)  # yb(t-SA) done reading B slot
                g.dma_start(out=B[s][:, :TILES[t]], in_=xo[:, sl]).then_inc(lsb[s], 16)

        @block.scalar
        def _(a):
            a.wait_ge(ksem, 16)
            ke = kk_sb[:, 0:1]
            ko = kk_sb[:, 1:2]
            for t in range(NTILES):
                s, f = t % SY, TILES[t]
                if t >= SY:
                    a.wait_ge(vo, t - SY + 1)  # add(t-SY) done reading ya/yb slots
                a.activation(
                    ya[s][:, :f], A[t % SA][:, :f], mybir.ActivationFunctionType.Lrelu,
                    bias=0.0, scale=ke, alpha=ALPHA,
                )._wait_ge(lsa[t % SA], 16 * (t // SA + 1)).then_inc(ay, 1)
                if t not in DVE_YB:
                    a.activation(
                        yb[s][:, :f], B[t % SA][:, :f], mybir.ActivationFunctionType.Lrelu,
                        bias=0.0, scale=ko, alpha=ALPHA,
                    )._wait_ge(lsb[t % SA], 16 * (t // SA + 1)).then_inc(byd, 1)

        @block.vector
        def _(v):
            ko = kk_sb[:, 1:2]
            for t in range(NTILES):
                s, f = t % SO, TILES[t]
                y_s = t % SY
                if t in DVE_YB:
                    # leaky via max(z, alpha*z); z = x * k_odd (per-partition)
                    v.tensor_scalar_mul(yb[y_s][:, :f], B[t % SA][:, :f], ko)._wait_ge(
                        lsb[t % SA], 16 * (t // SA + 1)
                    )
                    v.tensor_scalar_mul(tmp[:, :f], yb[y_s][:, :f], ALPHA)
                    v.tensor_max(yb[y_s][:, :f], yb[y_s][:, :f], tmp[:, :f]).then_inc(
                        byd, 1
                    )
                if t >= SO:
                    v.wait_ge(osem[s], 16 * ((t - SO) // SO + 1))
                v.wait_ge(ay, t + 1)
                v.tensor_add(
                    ob[s][:, :f], ya[y_s][:, :f], yb[y_s][:, :f]
                )._wait_ge(byd, t + 1).then_inc(vo, 1)

        @block.sync
        def _(ssy):
            ssy.dma_start(out=kk_sb, in_=kk[:]).then_inc(ksem, 16)
            for t in range(NTILES):
                s = t % SO
                sl = slice(OFFS[t], OFFS[t] + TILES[t])
                ssy.dma_start(out=out[:, sl], in_=ob[s][:, :TILES[t]])._wait_ge(
                    vo, t + 1
                ).then_inc(osem[s], 16)
            for sl in range(SO):
                n_dmas = NTILES // SO + (1 if sl < NTILES % SO else 0)
                ssy.wait_ge(osem[sl], 16 * n_dmas)

    nc.finalize()
    return nc


# which graph to use when copy_placer is all-zero: "trans", "lrelu" or "max01"
FAST_PATH = "trans"

# set by a test harness to capture a neuron-profile trace; harmless default
TRACE = False
LAST_RESULT = None

_GRAPH_CACHE = {}


def _get_graph(path):
    if path not in _GRAPH_CACHE:
        if path == "trans":
            _GRAPH_CACHE[path] = _build_transposed()
        elif path == "lrelu":
            _GRAPH_CACHE[path] = _build_lrelu()
        elif path == "max01":
            _GRAPH_CACHE[path] = _build_max(general=False)
        elif path == "maxgen":
            _GRAPH_CACHE[path] = _build_max(general=True)
        else:
            raise ValueError(path)
    return _GRAPH_CACHE[path]


def _rep(vec_f32):
    """(1, L) f32 -> (P, FD) bf16 tile with the vector repeated along free dim."""
    v = np.asarray(vec_f32, dtype=np.float32).reshape(1, L).astype(ml_dtypes.bfloat16)
    return np.ascontiguousarray(np.tile(v, (P, FD // L)))


def kernel(inputs, kernel, copy_placer):
    inputs = np.ascontiguousarray(inputs, dtype=np.float32)
    kv = np.asarray(kernel, dtype=np.float32).reshape(1, L)
    cv = np.asarray(copy_placer, dtype=np.float32).reshape(1, L)

    zero_copy = not np.any(cv)
    path = FAST_PATH if zero_copy else "maxgen"
    nc = _get_graph(path)

    in_maps = []
    if path == "trans":
        xb16 = inputs.astype(ml_dtypes.bfloat16)  # (BATCH, L)
        kkv = np.ascontiguousarray(
            kv.reshape(L // 2, 2).astype(np.float32)
        )  # row j -> [k[2j], k[2j+1]]
        for c in range(NCORES):
            xt = np.ascontiguousarray(xb16[c * SHARD:(c + 1) * SHARD].T)  # (L, SHARD)
            in_maps.append({
                "xe": np.ascontiguousarray(xt[0::2]),
                "xo": np.ascontiguousarray(xt[1::2]),
                "kk": kkv,
            })
    else:
        xs = inputs.reshape(NCORES, NT, P, FD)
        for c in range(NCORES):
            m = {"x": xs[c]}
            if path == "lrelu":
                m["kk"] = _rep(kv)
            elif path == "max01":
                m["khi"] = _rep(kv)
            else:
                m["khi"] = _rep(kv + cv)
                m["klo"] = _rep(ALPHA * kv + cv)
            in_maps.append(m)

    res = run_bass_kernel_spmd(nc, in_maps, list(range(NCORES)), trace=TRACE)
    global LAST_RESULT
    LAST_RESULT = res
    if path == "trans":
        full = np.empty((BATCH, L // 2), dtype=np.float32)
        for c in range(NCORES):
            oc = res.results[c]["out"].astype(np.float32)  # (128, SHARD)
            full[c * SHARD:(c + 1) * SHARD] = oc.T
        return full
    outs = [res.results[c]["out"].reshape(SHARD, L // 2) for c in range(NCORES)]
    return np.ascontiguousarray(np.concatenate(outs, axis=0))


if __name__ == "__main__":
    rng = np.random.default_rng(0)
    x = rng.standard_normal((BATCH, L)).astype(np.float32)
    k = (rng.standard_normal((1, L)) * np.sqrt(2.0)).astype(np.float32)
    c = np.zeros((1, L), dtype=np.float32)
    got = kernel(x, k, c)
    z = x * k
    y = np.where(z >= 0, z, ALPHA * z)
    exp = y[:, 0::2] + y[:, 1::2]
    err = np.abs(got - exp)
    print("max abs err:", err.max(), "absmax:", np.abs(exp).max())
    print("norm rel err:", np.linalg.norm((got - exp).ravel()) / np.linalg.norm(exp.ravel()))


# revision 12
# speedup vs baseline: 1.0268x; 1.0268x over previous
"""Trainium2 Bass kernel for nn_AsymmetricTreeLayer.

Computes out[b, j] = y[b, 2j] + y[b, 2j+1] where
  y = leaky_relu(inputs * kernel, 0.01) + inputs * copy_placer
for inputs (262144, 256) f32, kernel/copy_placer (1, 256) f32.

Strategy: pure data parallelism over 8 NeuronCores (batch sharded,
32768 rows per core). Per core the problem is memory bound
(32 MB in + 16 MB out at ~360 GB/s/core). Inputs are cast f32->bf16
during the DMA load (SWDGE cast), the per-column scale multiply runs on
VectorE at bf16 2x rate, leaky-relu on ScalarE (Lrelu activation), and
the pairwise reduction is a stride-2 tensor_tensor add writing f32.

Raw Bass (no Tile framework): this toolchain's walrus accepts at most
ONE sync-wait command per instruction, so all synchronization is manual
with standalone wait_ge instructions where more than one condition must
hold.
"""

import numpy as np
import sys

try:
    import concourse.bass as bass
except ImportError:  # staged repo location in the container
    sys.path.insert(0, "/opt/trn_rl_repo")
    import concourse.bass as bass
import ml_dtypes
import concourse.mybir as mybir
from concourse.bass_utils import run_bass_kernel_spmd

NCORES = 8
BATCH = 262144
L = 256
SHARD = BATCH // NCORES          # 32768 rows per core
P = 128                          # SBUF partitions
FD = 4096                        # f32 elems per partition per tile
ROWS_PER_PART = FD // L          # 16 rows of the (SHARD, L) shard per partition
NT = SHARD * L // (P * FD)       # 16 tiles per core
ALPHA = 0.01

BX = 3   # xb (input) buffer slots
BZ = 2   # z buffer slots
BY = 2   # y buffer slots
BO = 3   # output buffer slots


def _build_lrelu():
    """Fast path (copy_placer == 0): mul on DVE, Lrelu on ACT, pairwise on DVE."""
    nc = bass.Bass()
    x = nc.declare_dram_parameter("x", [NT, P, FD], mybir.dt.float32, isOutput=False)
    kk = nc.declare_dram_parameter("kk", [P, FD], mybir.dt.bfloat16, isOutput=False)
    out = nc.declare_dram_parameter("out", [NT, P, FD // 2], mybir.dt.float32, isOutput=True)

    k_sb = nc.alloc_sbuf_tensor("k_sb", [P, FD], mybir.dt.bfloat16).ap()
    xb = [nc.alloc_sbuf_tensor(f"xb{s}", [P, FD], mybir.dt.bfloat16).ap() for s in range(BX)]
    zb = [nc.alloc_sbuf_tensor(f"zb{s}", [P, FD], mybir.dt.bfloat16).ap() for s in range(BZ)]
    yb = [nc.alloc_sbuf_tensor(f"yb{s}", [P, FD], mybir.dt.bfloat16).ap() for s in range(BY)]
    ob = [nc.alloc_sbuf_tensor(f"ob{s}", [P, FD // 2], mybir.dt.float32).ap() for s in range(BO)]

    ksem = nc.alloc_semaphore("ksem")
    lsem = [nc.alloc_semaphore(f"lsem{s}") for s in range(BX)]
    osem = [nc.alloc_semaphore(f"osem{s}") for s in range(BO)]
    vz = nc.alloc_semaphore("vz")    # +1 per DVE mul
    ay = nc.alloc_semaphore("ay")    # +1 per ACT lrelu
    vo = nc.alloc_semaphore("vo")    # +1 per DVE pairwise

    with nc.Block() as block:

        @block.gpsimd
        def _(g):
            g.dma_start(out=k_sb, in_=kk[:]).then_inc(ksem, 16)
            for t in range(NT):
                if t >= BX:
                    # WAR: slot reused; its previous mul must be done
                    g.wait_ge(vz, t - BX + 1)
                g.dma_start(out=xb[t % BX], in_=x[t]).then_inc(lsem[t % BX], 16)

        @block.vector
        def _(v):
            def pairwise(j):
                if j >= BO:
                    v.wait_ge(osem[j % BO], 16 * (j // BO))
                y = yb[j % BY]
                v.tensor_add(ob[j % BO], y[:, 0::2], y[:, 1::2])._wait_ge(
                    ay, j + 1
                ).then_inc(vo, 1)

            v.wait_ge(ksem, 16)
            for t in range(NT):
                if t >= BZ:
                    v.wait_ge(ay, t - BZ + 1)  # WAR: lrelu done reading z slot
                v.tensor_mul(zb[t % BZ], xb[t % BX], k_sb)._wait_ge(
                    lsem[t % BX], 16 * (t // BX + 1)
                ).then_inc(vz, 1)
                if t > 0:
                    pairwise(t - 1)
            pairwise(NT - 1)

        @block.scalar
        def _(a):
            for t in range(NT):
                if t >= BY:
                    a.wait_ge(vo, t - BY + 1)  # WAR: pairwise done reading y slot
                a.activation(
                    yb[t % BY], zb[t % BZ], mybir.ActivationFunctionType.Lrelu,
                    bias=0.0, scale=1.0, alpha=ALPHA,
                )._wait_ge(vz, t + 1).then_inc(ay, 1)

        @block.sync
        def _(s):
            for t in range(NT):
                s.dma_start(out=out[t], in_=ob[t % BO])._wait_ge(vo, t + 1).then_inc(
                    osem[t % BO], 16
                )
            # drain: make sure every output DMA has landed before exit
            for sl in range(BO):
                n_dmas = NT // BO + (1 if sl < NT % BO else 0)
                s.wait_ge(osem[sl], 16 * n_dmas)

    nc.finalize()
    return nc


def _build_max(general: bool):
    """Fallback/general path on DVE only:
    out = pairwise(max(x*k_hi, x*k_lo)) with
      k_hi = kernel + copy_placer, k_lo = alpha*kernel + copy_placer.
    When general=False, k_lo tile is not loaded and b = a * alpha
    (valid for copy_placer == 0) via tensor_scalar at 4x rate.
    """
    nc = bass.Bass()
    x = nc.declare_dram_parameter("x", [NT, P, FD], mybir.dt.float32, isOutput=False)
    khi = nc.declare_dram_parameter("khi", [P, FD], mybir.dt.bfloat16, isOutput=False)
    if general:
        klo = nc.declare_dram_parameter("klo", [P, FD], mybir.dt.bfloat16, isOutput=False)
    out = nc.declare_dram_parameter("out", [NT, P, FD // 2], mybir.dt.float32, isOutput=True)

    khi_sb = nc.alloc_sbuf_tensor("khi_sb", [P, FD], mybir.dt.bfloat16).ap()
    if general:
        klo_sb = nc.alloc_sbuf_tensor("klo_sb", [P, FD], mybir.dt.bfloat16).ap()
    xb = [nc.alloc_sbuf_tensor(f"xb{s}", [P, FD], mybir.dt.bfloat16).ap() for s in range(BX)]
    ab = [nc.alloc_sbuf_tensor(f"ab{s}", [P, FD], mybir.dt.bfloat16).ap() for s in range(2)]
    bb = [nc.alloc_sbuf_tensor(f"bb{s}", [P, FD], mybir.dt.bfloat16).ap() for s in range(2)]
    yb = [nc.alloc_sbuf_tensor(f"yb{s}", [P, FD], mybir.dt.bfloat16).ap() for s in range(2)]
    ob = [nc.alloc_sbuf_tensor(f"ob{s}", [P, FD // 2], mybir.dt.float32).ap() for s in range(BO)]

    ksem = nc.alloc_semaphore("ksem")
    lsem = [nc.alloc_semaphore(f"lsem{s}") for s in range(BX)]
    osem = [nc.alloc_semaphore(f"osem{s}") for s in range(BO)]
    vz = nc.alloc_semaphore("vz")    # +1 per a-mul (for load WAR)
    vo = nc.alloc_semaphore("vo")    # +1 per pairwise

    with nc.Block() as block:

        @block.gpsimd
        def _(g):
            g.dma_start(out=khi_sb, in_=khi[:]).then_inc(ksem, 16)
            if general:
                g.dma_start(out=klo_sb, in_=klo[:]).then_inc(ksem, 16)
            for t in range(NT):
                if t >= BX:
                    g.wait_ge(vz, t - BX + 1)
                g.dma_start(out=xb[t % BX], in_=x[t]).then_inc(lsem[t % BX], 16)

        @block.vector
        def _(v):
            v.wait_ge(ksem, 32 if general else 16)
            for t in range(NT):
                a, b, y = ab[t % 2], bb[t % 2], yb[t % 2]
                v.tensor_mul(a, xb[t % BX], khi_sb)._wait_ge(
                    lsem[t % BX], 16 * (t // BX + 1)
                ).then_inc(vz, 1)
                if general:
                    v.tensor_mul(b, xb[t % BX], klo_sb)
                else:
                    v.tensor_scalar_mul(b, a, ALPHA)
                v.tensor_max(y, a, b)
                if t >= BO:
                    v.wait_ge(osem[t % BO], 16 * (t // BO))
                v.tensor_add(ob[t % BO], y[:, 0::2], y[:, 1::2]).then_inc(vo, 1)

        @block.sync
        def _(s):
            for t in range(NT):
                s.dma_start(out=out[t], in_=ob[t % BO])._wait_ge(vo, t + 1).then_inc(
                    osem[t % BO], 16
                )
            for sl in range(BO):
                n_dmas = NT // BO + (1 if sl < NT % BO else 0)
                s.wait_ge(osem[sl], 16 * n_dmas)

    nc.finalize()
    return nc


FT = 4096                        # max batch elems per partition per tile
# tile sizes along the batch axis; tapered tail shortens the drain chain
TILES = [4096] * 7 + [2048, 1024, 1024]
assert sum(TILES) == SHARD
# tiles whose odd-half leaky runs on VectorE (max-trick) instead of ScalarE,
# balancing the two engines so neither paces the pipeline tail
DVE_YB = {1, 3, 5, 6, 7, 8, 9}
SA = 3                           # input (A/B) buffer slots
SY = 2                           # ya/yb buffer slots
SO = 3                           # output buffer slots


def _build_transposed():
    """Transposed fast path (copy_placer == 0).

    Host pre-transposes each core's shard to column-major bf16 and splits
    even/odd columns: xe[j, b] = x[b, 2j], xo[j, b] = x[b, 2j+1].
    Then k becomes a per-partition scale, so ScalarE computes
    y = Lrelu(k_j * x) in a single activation op per half, and VectorE
    only does out[j, b] = ye + yo (contiguous bf16 add at 2x rate).
    Output is bf16 (128, SHARD), transposed back and upcast on the host.
    """
    nc = bass.Bass()
    xe = nc.declare_dram_parameter("xe", [P, SHARD], mybir.dt.bfloat16, isOutput=False)
    xo = nc.declare_dram_parameter("xo", [P, SHARD], mybir.dt.bfloat16, isOutput=False)
    kk = nc.declare_dram_parameter("kk", [P, 2], mybir.dt.float32, isOutput=False)
    out = nc.declare_dram_parameter("out", [P, SHARD], mybir.dt.bfloat16, isOutput=True)

    NTILES = len(TILES)
    OFFS = [sum(TILES[:i]) for i in range(NTILES)]

    kk_sb = nc.alloc_sbuf_tensor("kk_sb", [P, 2], mybir.dt.float32).ap()
    A = [nc.alloc_sbuf_tensor(f"A{s}", [P, FT], mybir.dt.bfloat16).ap() for s in range(SA)]
    B = [nc.alloc_sbuf_tensor(f"B{s}", [P, FT], mybir.dt.bfloat16).ap() for s in range(SA)]
    ya = [nc.alloc_sbuf_tensor(f"ya{s}", [P, FT], mybir.dt.bfloat16).ap() for s in range(SY)]
    yb = [nc.alloc_sbuf_tensor(f"yb{s}", [P, FT], mybir.dt.bfloat16).ap() for s in range(SY)]
    ob = [nc.alloc_sbuf_tensor(f"ob{s}", [P, FT], mybir.dt.bfloat16).ap() for s in range(SO)]
    tmp = nc.alloc_sbuf_tensor("tmp", [P, FT], mybir.dt.bfloat16).ap()

    ksem = nc.alloc_semaphore("ksem")
    lsa = [nc.alloc_semaphore(f"lsa{s}") for s in range(SA)]
    lsb = [nc.alloc_semaphore(f"lsb{s}") for s in range(SA)]
    osem = [nc.alloc_semaphore(f"osem{s}") for s in range(SO)]
    ay = nc.alloc_semaphore("ay")    # +1 per ya (ScalarE)
    byd = nc.alloc_semaphore("byd")  # +1 per yb, whichever engine produced it
    vo = nc.alloc_semaphore("vo")    # +1 per DVE pairwise add

    with nc.Block() as block:

        @block.gpsimd
        def _(g):
            for t in range(NTILES):
                s = t % SA
                sl = slice(OFFS[t], OFFS[t] + TILES[t])
                if t >= SA:
                    g.wait_ge(ay, t - SA + 1)   # ya(t-SA) done reading A slot
                g.dma_start(out=A[s][:, :TILES[t]], in_=xe[:, sl]).then_inc(lsa[s], 16)
                if t >= SA:
                    g.wait_ge(byd, t - SA + 1)  # yb(t-SA) done reading B slot
                g.dma_start(out=B[s][:, :TILES[t]], in_=xo[:, sl]).then_inc(lsb[s], 16)

        @block.scalar
        def _(a):
            a.wait_ge(ksem, 16)
            ke = kk_sb[:, 0:1]
            ko = kk_sb[:, 1:2]
            for t in range(NTILES):
                s, f = t % SY, TILES[t]
                if t >= SY:
                    a.wait_ge(vo, t - SY + 1)  # add(t-SY) done reading ya/yb slots
                a.activation(
                    ya[s][:, :f], A[t % SA][:, :f], mybir.ActivationFunctionType.Lrelu,
                    bias=0.0, scale=ke, alpha=ALPHA,
                )._wait_ge(lsa[t % SA], 16 * (t // SA + 1)).then_inc(ay, 1)
                if t not in DVE_YB:
                    a.activation(
                        yb[s][:, :f], B[t % SA][:, :f], mybir.ActivationFunctionType.Lrelu,
                        bias=0.0, scale=ko, alpha=ALPHA,
                    )._wait_ge(lsb[t % SA], 16 * (t // SA + 1)).then_inc(byd, 1)

        @block.vector
        def _(v):
            ko = kk_sb[:, 1:2]
            for t in range(NTILES):
                s, f = t % SO, TILES[t]
                y_s = t % SY
                if t in DVE_YB:
                    # leaky via max(z, alpha*z); z = x * k_odd (per-partition)
                    v.tensor_scalar_mul(yb[y_s][:, :f], B[t % SA][:, :f], ko)._wait_ge(
                        lsb[t % SA], 16 * (t // SA + 1)
                    )
                    v.tensor_scalar_mul(tmp[:, :f], yb[y_s][:, :f], ALPHA)
                    v.tensor_max(yb[y_s][:, :f], yb[y_s][:, :f], tmp[:, :f]).then_inc(
                        byd, 1
                    )
                if t >= SO:
                    v.wait_ge(osem[s], 16 * ((t - SO) // SO + 1))
                v.wait_ge(ay, t + 1)
                v.tensor_add(
                    ob[s][:, :f], ya[y_s][:, :f], yb[y_s][:, :f]
                )._wait_ge(byd, t + 1).then_inc(vo, 1)

        @block.sync
        def _(ssy):
            ssy.dma_start(out=kk_sb, in_=kk[:]).then_inc(ksem, 16)
            for t in range(NTILES):
                s = t % SO
                sl = slice(OFFS[t], OFFS[t] + TILES[t])
                ssy.dma_start(out=out[:, sl], in_=ob[s][:, :TILES[t]])._wait_ge(
                    vo, t + 1
                ).then_inc(osem[s], 16)
            for sl in range(SO):
                n_dmas = NTILES // SO + (1 if sl < NTILES % SO else 0)
                ssy.wait_ge(osem[sl], 16 * n_dmas)

    nc.finalize()
    return nc


# which graph to use when copy_placer is all-zero: "trans", "lrelu" or "max01"
FAST_PATH = "trans"

# set by a test harness to capture a neuron-profile trace; harmless default
TRACE = False
LAST_RESULT = None

_GRAPH_CACHE = {}


def _get_graph(path):
    if path not in _GRAPH_CACHE:
        if path == "trans":
            _GRAPH_CACHE[path] = _build_transposed()
        elif path == "lrelu":
            _GRAPH_CACHE[path] = _build_lrelu()
        elif path == "max01":
            _GRAPH_CACHE[path] = _build_max(general=False)
        elif path == "maxgen":
            _GRAPH_CACHE[path] = _build_max(general=True)
        else:
            raise ValueError(path)
    return _GRAPH_CACHE[path]


def _rep(vec_f32):
    """(1, L) f32 -> (P, FD) bf16 tile with the vector repeated along free dim."""
    v = np.asarray(vec_f32, dtype=np.float32).reshape(1, L).astype(ml_dtypes.bfloat16)
    return np.ascontiguousarray(np.tile(v, (P, FD // L)))


def kernel(inputs, kernel, copy_placer):
    inputs = np.ascontiguousarray(inputs, dtype=np.float32)
    kv = np.asarray(kernel, dtype=np.float32).reshape(1, L)
    cv = np.asarray(copy_placer, dtype=np.float32).reshape(1, L)

    zero_copy = not np.any(cv)
    path = FAST_PATH if zero_copy else "maxgen"
    nc = _get_graph(path)

    in_maps = []
    if path == "trans":
        xb16 = inputs.astype(ml_dtypes.bfloat16)  # (BATCH, L)
        kkv = np.ascontiguousarray(
            kv.reshape(L // 2, 2).astype(np.float32)
        )  # row j -> [k[2j], k[2j+1]]
        for c in range(NCORES):
            xt = np.ascontiguousarray(xb16[c * SHARD:(c + 1) * SHARD].T)  # (L, SHARD)
            in_maps.append({
                "xe": np.ascontiguousarray(xt[0::2]),
                "xo": np.ascontiguousarray(xt[1::2]),
                "kk": kkv,
            })
    else:
        xs = inputs.reshape(NCORES, NT, P, FD)
        for c in range(NCORES):
            m = {"x": xs[c]}
            if path == "lrelu":
                m["kk"] = _rep(kv)
            elif path == "max01":
                m["khi"] = _rep(kv)
            else:
                m["khi"] = _rep(kv + cv)
                m["klo"] = _rep(ALPHA * kv + cv)
            in_maps.append(m)

    res = run_bass_kernel_spmd(nc, in_maps, list(range(NCORES)), trace=TRACE)
    global LAST_RESULT
    LAST_RESULT = res
    if path == "trans":
        full = np.empty((BATCH, L // 2), dtype=np.float32)
        for c in range(NCORES):
            oc = res.results[c]["out"].astype(np.float32)  # (128, SHARD)
            full[c * SHARD:(c + 1) * SHARD] = oc.T
        return full
    outs = [res.results[c]["out"].reshape(SHARD, L // 2) for c in range(NCORES)]
    return np.ascontiguousarray(np.concatenate(outs, axis=0))


if __name__ == "__main__":
    rng = np.random.default_rng(0)
    x = rng.standard_normal((BATCH, L)).astype(np.float32)
    k = (rng.standard_normal((1, L)) * np.sqrt(2.0)).astype(np.float32)
    c = np.zeros((1, L), dtype=np.float32)
    got = kernel(x, k, c)
    z = x * k
    y = np.where(z >= 0, z, ALPHA * z)
    exp = y[:, 0::2] + y[:, 1::2]
    err = np.abs(got - exp)
    print("max abs err:", err.max(), "absmax:", np.abs(exp).max())
    print("norm rel err:", np.linalg.norm((got - exp).ravel()) / np.linalg.norm(exp.ravel()))
